# revision 13
# baseline (speedup 1.0000x reference)
"""GraphSAGE (mean) 3-layer encoder on 8 Trainium2 NeuronCores.

Strategy (graph/data parallel, per sharding hint):
  - Nodes sharded contiguously across 8 cores (12500/core, padded to
    12544 = 98*128 "slots"); per-core nodes permuted by in-degree so
    per-(group,bucket) edge counts correlate across cores (less chunk
    padding under SPMD's shared chunk layout).
  - Layer 0 does NO per-edge gather at all: since e = emb[x] with a
    5000-row vocabulary, the mean aggregation is neigh0 = C @ emb with
    C[v, slot] = sum of 1/deg over edges (u -> slot) with x[u] = v.
    C is host-built (fp8e4, dense over the vocab) and streamed per
    group; emb stays resident in SBUF (bf16). 40 mixed bf16 x fp8
    matmuls per group.
  - Layers 1-2 route edges by dst core on the host, sorted by
    (dst group of 4 blocks, src bucket, slot-in-group). dma_gather
    (int16 indices over 4 x 25088-row buckets) fetches src rows in
    128-edge chunks; the dst-selector one-hots (layer-invariant, 1/deg
    folded, bf16) are host-built and streamed -- per chunk only the
    128-col subblocks its edges touch (union across cores, so the SPMD
    program is shared). PE matmuls (gathered^T @ onehot) accumulate the
    mean-aggregated neighborhood feature-major in a per-group PSUM bank
    with per-subcolumn start/stop flags.
  - Dense part per group: bf16 matmuls (self + neigh into one PSUM),
    bias+ReLU (ACT), PE transpose, L2 norm + residual (ACT/DVE), bf16
    table DMA -> AllGather rebuilds the global feature table (after
    layers 0 and 1).
"""

import math
import sys

import numpy as np

for _p in ("/opt/trn_rl_repo", "/root/.axon_site/_ro/trn_rl_repo"):
    if _p not in sys.path:
        sys.path.append(_p)

import concourse.bacc as bacc  # noqa: E402
import concourse.bass as bass  # noqa: E402
import concourse.mybir as mybir  # noqa: E402
import concourse.tile as tile  # noqa: E402
from concourse import bass_utils  # noqa: E402
from concourse.masks import make_identity  # noqa: E402

M = 8  # cores
D = 128
P = 128
NBUC = 4  # src buckets (int16 index range)
GRP = 4  # dst blocks per group

LAST_EXEC_NS = None  # set by kernel() when _trace=True


def _host_prep(x, src, dst, n_nodes, V):
    N = n_nodes
    NPC = math.ceil(N / M)
    SLOTS = math.ceil(NPC / P) * P
    NBLK = SLOTS // P
    TBL = M * SLOTS
    BUC = TBL // NBUC
    assert BUC * NBUC == TBL and BUC <= 32768
    NG = math.ceil(NBLK / GRP)
    VP = math.ceil(V / P) * P
    NVC = VP // P

    x = np.asarray(x).astype(np.int64)
    src = np.asarray(src).astype(np.int64)
    dst = np.asarray(dst).astype(np.int64)
    E = len(src)

    deg = np.bincount(dst, minlength=N)
    invd = (1.0 / np.maximum(deg, 1.0)).astype(np.float64)
    core_of_node = np.minimum(np.arange(N) // NPC, M - 1)
    perm = np.empty(N, np.int64)
    for c in range(M):
        lo, hi = c * NPC, min((c + 1) * NPC, N)
        nodes = np.arange(lo, hi)
        order = np.argsort(deg[nodes], kind="stable")
        r = np.empty(len(nodes), np.int64)
        r[order] = np.arange(len(nodes))
        perm[nodes] = r
    gslot = core_of_node * SLOTS + perm

    gof = np.minimum(np.arange(NBLK) // GRP, NG - 1)  # group of block
    pof = np.arange(NBLK) - np.searchsorted(
        np.arange(NBLK)[np.r_[True, np.diff(gof) > 0]], np.arange(NBLK), "right"
    )  # not used; compute pos-in-group directly below
    groups = [list(range(g * GRP, min((g + 1) * GRP, NBLK))) for g in range(NG)]
    gw = [len(g) * P for g in groups]
    posof = np.empty(NBLK, np.int64)
    for g, js in enumerate(groups):
        for pi, j in enumerate(js):
            posof[j] = pi

    # per-core edge arrays sorted by (group, bucket, gslot4)
    ecore = core_of_node[dst]
    cores_edges = []
    cnt_cgb = np.zeros((M, NG, NBUC), np.int64)
    for c in range(M):
        sel = ecore == c
        dslot = perm[dst[sel]]
        sg = gslot[src[sel]]
        wv = invd[dst[sel]]
        blk = dslot // P
        gg = gof[blk]
        g4 = posof[blk] * P + dslot % P
        b = sg // BUC
        o = np.lexsort((g4, b, gg))
        cores_edges.append((gg[o], b[o], g4[o], sg[o], wv[o]))
        np.add.at(cnt_cgb[c], (gg, b), 1)

    C_gb = np.ceil(cnt_cgb / P).astype(np.int64).max(axis=0)  # [NG, NBUC]
    for g in range(NG):
        if C_gb[g].sum() == 0:
            C_gb[g, 0] = 1

    # chunk stream layout: for g: for b: chunks
    chcol = np.zeros((NG, NBUC), np.int64)
    pos = 0
    for g in range(NG):
        for b in range(NBUC):
            chcol[g, b] = pos
            pos += int(C_gb[g, b])
    NCH = pos
    NIDX = NCH * P

    # per-chunk subblock sets (union over cores) -> entries
    sub_sets = [set() for _ in range(NCH)]
    per_core_edge = []
    for c in range(M):
        gg, b, g4, sg, wv = cores_edges[c]
        flat = gg * NBUC + b
        cnts = cnt_cgb[c].reshape(-1)
        st = np.zeros(NG * NBUC, np.int64)
        st[1:] = np.cumsum(cnts)[:-1]
        rank = np.arange(len(gg)) - st[flat]
        ch = chcol[gg, b] + rank // P
        pp = rank % P
        per_core_edge.append((ch, pp, g4, sg, b, wv))
        key = ch * 8 + g4 // P
        for k in np.unique(key):
            sub_sets[int(k) // 8].add(int(k) % 8)

    gof_ch = np.zeros(NCH, np.int64)
    for g in range(NG):
        for b in range(NBUC):
            gof_ch[chcol[g, b] : chcol[g, b] + C_gb[g, b]] = g
    # ensure each (g, subcol) with no entry gets one dummy entry so PSUM
    # is initialized before the dense matmul reads it
    touched = [set() for _ in range(NG)]
    for ch in range(NCH):
        for sc in sub_sets[ch]:
            touched[gof_ch[ch]].add(sc)
    for g in range(NG):
        for sc in range(gw[g] // P):
            if sc not in touched[g]:
                first_ch = chcol[g, 0]
                sub_sets[first_ch].add(sc)

    # entry list: for each chunk (stream order), sorted subcols
    entries = []  # (ch, sc, colpos)
    entry_col = {}
    colpos = 0
    for ch in range(NCH):
        for sc in sorted(sub_sets[ch]):
            entries.append((ch, sc, colpos))
            entry_col[(ch, sc)] = colpos
            colpos += P
    OHC = colpos

    # per-group ordered entry lists; PSUM is opened by one bank-wide
    # zeroing matmul per group, so every entry accumulates (start=False)
    # and only the last one carries stop=True
    group_entries = [[] for _ in range(NG)]
    for (ch, sc, cp) in entries:
        group_entries[gof_ch[ch]].append([ch, sc, cp, False, False])
    for g in range(NG):
        assert group_entries[g], f"group {g} has no aggregation entries"
        group_entries[g][-1][4] = True  # stop

    # per-(g,b) oh slab column ranges (contiguous in stream order)
    slab = {}
    for g in range(NG):
        for b in range(NBUC):
            c0, c1 = chcol[g, b], chcol[g, b] + C_gb[g, b]
            cols = [cp for (ch, sc, cp) in entries if c0 <= ch < c1]
            if cols:
                slab[(g, b)] = (min(cols), max(cols) + P)
            else:
                slab[(g, b)] = (0, 0)

    per_core = []
    for c in range(M):
        ch, pp, g4, sg, b, wv = per_core_edge[c]
        idxs = np.zeros(NIDX, np.int16)
        idxs[ch * P + pp] = (sg - b * BUC).astype(np.int16)
        oh = np.zeros((P, OHC), np.float32)
        ecol = np.array(
            [entry_col[(int(cc), int(ss))] for cc, ss in zip(ch, g4 // P)],
            np.int64,
        )
        oh[pp, ecol + g4 % P] = wv

        idx16 = idxs.reshape(NIDX // 16, 16).T.copy()
        idx_full = np.tile(idx16, (8, 1))

        # layer-0 C matrix: [VP, SLOTS] -> slabs [P, sum_g NVC*gw]
        Cf = np.zeros((VP, SLOTS), np.float32)
        sel = ecore == c
        np.add.at(Cf, (x[src[sel]], perm[dst[sel]]), invd[dst[sel]])
        cslabs = []
        for g, js in enumerate(groups):
            blkcols = np.concatenate([np.arange(j * P, (j + 1) * P) for j in js])
            Cg = Cf[:, blkcols].reshape(NVC, P, gw[g])  # [k, vpart, gwcols]
            cslabs.append(np.transpose(Cg, (1, 0, 2)).reshape(P, NVC * gw[g]))
        cmat = np.concatenate(cslabs, axis=1)  # [P, NVC * sum gw]

        per_core.append({"gidx": idx_full, "oh": oh, "cmat": cmat})

    ccol = np.zeros(NG + 1, np.int64)
    for g in range(NG):
        ccol[g + 1] = ccol[g] + NVC * gw[g]

    meta = {
        "NPC": NPC,
        "SLOTS": SLOTS,
        "NBLK": NBLK,
        "TBL": TBL,
        "BUC": BUC,
        "NG": NG,
        "VP": VP,
        "NVC": NVC,
        "groups": groups,
        "gw": gw,
        "chcol": chcol,
        "C_gb": C_gb,
        "group_entries": group_entries,
        "slab": slab,
        "NCH": NCH,
        "NIDX": NIDX,
        "OHC": OHC,
        "ccol": ccol,
        "gslot": gslot,
    }
    return per_core, meta


def _build_program(meta, L):
    SLOTS, NBLK, TBL, BUC = meta["SLOTS"], meta["NBLK"], meta["TBL"], meta["BUC"]
    NG, VP, NVC = meta["NG"], meta["VP"], meta["NVC"]
    groups, gw, chcol, C_gb = meta["groups"], meta["gw"], meta["chcol"], meta["C_gb"]
    group_entries, slab = meta["group_entries"], meta["slab"]
    NCH, NIDX, OHC, ccol = meta["NCH"], meta["NIDX"], meta["OHC"], meta["ccol"]
    CBMAX = int(C_gb.max())
    SLABMAX = max(hi - lo for (lo, hi) in slab.values())
    CHALF = NVC // 2

    f32, bf16 = mybir.dt.float32, mybir.dt.bfloat16
    i16 = mybir.dt.int16

    nc = bacc.Bacc(
        "TRN2",
        target_bir_lowering=False,
        debug=False,
        enable_asserts=False,
        num_devices=M,
    )

    gidx_d = nc.dram_tensor("gidx", [P, NIDX // 16], i16, kind="ExternalInput")
    oh_d = nc.dram_tensor("oh", [P, OHC], bf16, kind="ExternalInput")
    cmat_d = nc.dram_tensor(
        "cmat", [P, int(ccol[-1])], mybir.dt.float8e4, kind="ExternalInput"
    )
    emb_d = nc.dram_tensor("emb", [VP, D], bf16, kind="ExternalInput")
    esh_d = nc.dram_tensor("esh", [SLOTS, D], bf16, kind="ExternalInput")
    ws_d = nc.dram_tensor("ws", [L, D, D], bf16, kind="ExternalInput")
    wn_d = nc.dram_tensor("wn", [L, D, D], bf16, kind="ExternalInput")
    bias_d = nc.dram_tensor("bias", [L, D], f32, kind="ExternalInput")
    hout_d = nc.dram_tensor("hout", [SLOTS, D], bf16, kind="ExternalOutput")

    h_shard = nc.dram_tensor("h_shard", [SLOTS, D], bf16, kind="Internal")
    h_full = nc.dram_tensor(
        "h_full", [TBL, D], bf16, kind="Internal", addr_space="Shared"
    )

    rg = [list(range(M))]

    with tile.TileContext(nc) as tc:
        with (
            tc.tile_pool(name="const", bufs=1) as cpool,
            tc.tile_pool(name="state", bufs=1) as spool,
            tc.tile_pool(name="gath", bufs=5) as gpool,
            tc.tile_pool(name="oht", bufs=4) as ohpool,
            tc.tile_pool(name="cslab", bufs=2) as cpool2,
            tc.tile_pool(name="fm", bufs=2) as fmpool,
            tc.tile_pool(name="small", bufs=3) as smpool,
            tc.tile_pool(name="ps_agg", bufs=2, space="PSUM") as ps_agg,
            tc.tile_pool(name="ps_tp", bufs=2, space="PSUM") as ps_tp,
            tc.tile_pool(name="ps_nm", bufs=2, space="PSUM") as ps_nm,
            tc.tile_pool(name="ps_d", bufs=2, space="PSUM") as ps_d,
        ):
            # ---- constants ----
            ident_bf = cpool.tile([P, P], bf16, tag="ident_bf")
            make_identity(nc, ident_bf[:])
            zt = cpool.tile([1, P], bf16, tag="zt")
            nc.vector.memset(zt[:], 0)
            rz = cpool.tile([1, GRP * D], bf16, tag="rz")
            nc.vector.memset(rz[:], 0)

            gidx_sb = cpool.tile([P, NIDX // 16], i16, tag="gidx")
            nc.sync.dma_start(gidx_sb[:], gidx_d[:, :])

            emb_sb = cpool.tile([P, NVC * D], bf16, tag="emb")
            emb_v = emb_d.ap().rearrange("(k p) f -> p k f", p=P)
            nc.sync.dma_start(emb_sb[:].rearrange("p (k f) -> p k f", f=D), emb_v)

            w_sb = []
            for l in range(L):
                ws = cpool.tile([P, D], bf16, tag=f"ws{l}")
                wn = cpool.tile([P, D], bf16, tag=f"wn{l}")
                nc.sync.dma_start(ws[:], ws_d[l, :, :])
                nc.sync.dma_start(wn[:], wn_d[l, :, :])
                w_sb.append((ws, wn))
            b_sb = cpool.tile([P, L], f32, tag="bias")
            for l in range(L):
                nc.sync.dma_start(b_sb[:, l : l + 1], bias_d[l, :, None])

            # ---- e shard (node-major bf16) ----
            e_sb = spool.tile([P, NBLK * D], bf16, tag="e")
            esh_v = esh_d.ap().rearrange("(j p) f -> p j f", p=P)
            nc.sync.dma_start(e_sb[:].rearrange("p (j f) -> p j f", f=D), esh_v)

            h_sb = spool.tile([P, NBLK * D], bf16, tag="h")

            shard_v = h_shard.ap().rearrange("(j p) f -> p j f", p=P)

            def store_table(src_tile):
                sv = src_tile[:].rearrange("p (j f) -> p j f", f=D)
                nc.sync.dma_start(shard_v, sv)
                nc.gpsimd.collective_compute(
                    "AllGather",
                    mybir.AluOpType.bypass,
                    replica_groups=rg,
                    ins=[h_shard[:, :]],
                    outs=[h_full[:, :]],
                )

            def dense_and_post(l, gi, grp, cur, nfm_src):
                """nfm_src: PSUM tile [P, gw] with feature-major neigh."""
                ws, wn = w_sb[l]
                gwg = gw[gi]
                nfm = fmpool.tile([P, GRP * D], bf16, tag="nfm")
                nc.scalar.copy(nfm[:, 0:gwg], nfm_src[:, 0:gwg])
                hfm = fmpool.tile([P, GRP * D], bf16, tag="hfm")
                for bi, j in enumerate(grp):
                    pt = ps_tp.tile([P, P], bf16, tag="tp")
                    nc.tensor.transpose(
                        pt[:], cur[:, j * D : (j + 1) * D], ident_bf[:]
                    )
                    nc.scalar.copy(hfm[:, bi * D : (bi + 1) * D], pt[:])
                pd = ps_d.tile([P, GRP * D], f32, tag="d")
                nc.tensor.matmul(
                    pd[:, 0:gwg], ws[:], hfm[:, 0:gwg], start=True, stop=False
                )
                nc.tensor.matmul(
                    pd[:, 0:gwg], wn[:], nfm[:, 0:gwg], start=False, stop=True
                )
                hpre = fmpool.tile([P, GRP * D], bf16, tag="hpre")
                nc.scalar.activation(
                    hpre[:, 0:gwg],
                    pd[:, 0:gwg],
                    mybir.ActivationFunctionType.Relu,
                    bias=b_sb[:, l : l + 1],
                )
                for bi, j in enumerate(grp):
                    pn = ps_nm.tile([P, P], bf16, tag="nm")
                    nc.tensor.transpose(
                        pn[:], hpre[:, bi * D : (bi + 1) * D], ident_bf[:]
                    )
                    sq = smpool.tile([P, D], f32, tag="sq")
                    ss = smpool.tile([P, 1], f32, tag="ss")
                    nc.scalar.activation(
                        sq[:],
                        pn[:],
                        mybir.ActivationFunctionType.Square,
                        accum_out=ss[:],
                    )
                    nrm = smpool.tile([P, 1], f32, tag="nrm")
                    nc.scalar.sqrt(nrm[:], ss[:])
                    nc.vector.tensor_scalar_max(nrm[:], nrm[:], 1e-12)
                    inv = smpool.tile([P, 1], f32, tag="inv")
                    nc.vector.reciprocal(inv[:], nrm[:])
                    htmp = smpool.tile([P, D], f32, tag="htmp")
                    nc.vector.tensor_scalar(
                        htmp[:], pn[:], inv[:], None, mybir.AluOpType.mult
                    )
                    nc.vector.tensor_tensor(
                        out=h_sb[:, j * D : (j + 1) * D],
                        in0=htmp[:],
                        in1=e_sb[:, j * D : (j + 1) * D],
                        op=mybir.AluOpType.add,
                    )

            # ---- layer 0: vocabulary-space aggregation ----
            for gi, grp in enumerate(groups):
                gwg = gw[gi]
                pa = ps_agg.tile([P, GRP * D], f32, tag="agg")
                for half in range(2):
                    k0 = half * CHALF
                    cs = cpool2.tile([P, CHALF * GRP * D], mybir.dt.float8e4, tag="cs")
                    nc.sync.dma_start(
                        cs[:, 0 : CHALF * gwg],
                        cmat_d[:, int(ccol[gi]) + k0 * gwg : int(ccol[gi]) + (k0 + CHALF) * gwg],
                    )
                    for kk in range(CHALF):
                        k = k0 + kk
                        nc.tensor.matmul(
                            pa[:, 0:gwg],
                            emb_sb[:, k * D : (k + 1) * D],
                            cs[:, kk * gwg : (kk + 1) * gwg],
                            start=(k == 0),
                            stop=(k == NVC - 1),
                        )
                dense_and_post(0, gi, grp, e_sb, pa)
            store_table(h_sb)

            # ---- layers 1..L-1: gather + one-hot aggregation ----
            for l in range(1, L):
                cur = h_sb
                for gi, grp in enumerate(groups):
                    gwg = gw[gi]
                    gtiles = {}
                    ohtiles = {}
                    for b in range(NBUC):
                        nchb = int(C_gb[gi, b])
                        if nchb == 0:
                            continue
                        ch0 = int(chcol[gi, b])
                        gt = gpool.tile([P, CBMAX, D], bf16, tag="gath")
                        ni = nchb * P
                        nc.gpsimd.dma_gather(
                            gt[:, 0:nchb, :],
                            h_full[b * BUC : (b + 1) * BUC, :],
                            gidx_sb[:, ch0 * 8 : (ch0 + nchb) * 8],
                            ni,
                            ni,
                            D,
                            single_packet=False,
                        )
                        gtiles[b] = (gt, ch0)
                        lo, hi = slab[(gi, b)]
                        if hi > lo:
                            ot = ohpool.tile([P, SLABMAX], bf16, tag="oht")
                            nc.sync.dma_start(ot[:, 0 : hi - lo], oh_d[:, lo:hi])
                            ohtiles[b] = (ot, lo)
                    pa = ps_agg.tile([P, GRP * D], f32, tag="agg")
                    nc.tensor.matmul(
                        pa[:, 0:gwg], zt[:, :], rz[:, 0:gwg], start=True, stop=False
                    )
                    for (ch, sc, cp, st, sp) in group_entries[gi]:
                        b = int(np.searchsorted(chcol[gi], ch, "right")) - 1
                        gt, ch0 = gtiles[b]
                        ot, lo = ohtiles[b]
                        nc.tensor.matmul(
                            pa[:, sc * P : (sc + 1) * P],
                            gt[:, ch - ch0, :],
                            ot[:, cp - lo : cp - lo + P],
                            start=False,
                            stop=sp,
                        )
                    dense_and_post(l, gi, grp, cur, pa)
                if l < L - 1:
                    store_table(h_sb)

            hout_v = hout_d.ap().rearrange("(j p) f -> p j f", p=P)
            h_v = h_sb[:].rearrange("p (j f) -> p j f", f=D)
            nc.sync.dma_start(hout_v, h_v)

    nc.compile()
    return nc


def kernel(x, src, dst, emb, Ws, Wn, b, _trace=False):
    import jax.numpy as jnp

    x = np.asarray(x)
    src = np.asarray(src)
    dst = np.asarray(dst)
    emb = np.ascontiguousarray(np.asarray(emb, dtype=np.float32))
    Ws = np.ascontiguousarray(np.asarray(Ws, dtype=np.float32))
    Wn = np.ascontiguousarray(np.asarray(Wn, dtype=np.float32))
    b = np.ascontiguousarray(np.asarray(b, dtype=np.float32))
    N = x.shape[0]
    V = emb.shape[0]
    L = Ws.shape[0]

    emb_bf = np.asarray(jnp.asarray(emb, jnp.bfloat16))
    ws_bf = np.asarray(jnp.asarray(Ws, jnp.bfloat16))
    wn_bf = np.asarray(jnp.asarray(Wn, jnp.bfloat16))

    per_core, meta = _host_prep(x, src, dst, N, V)
    nc = _build_program(meta, L)

    SLOTS, VP = meta["SLOTS"], meta["VP"]
    gslot = meta["gslot"]
    emb_pad = np.zeros((VP, D), emb_bf.dtype)
    emb_pad[:V] = emb_bf
    # e table rows in global slot order for the residual shards
    e_full = np.zeros((meta["TBL"], D), emb_bf.dtype)
    e_full[gslot] = emb_bf[x]

    in_maps = []
    for c in range(M):
        pc = per_core[c]
        in_maps.append(
            {
                "gidx": np.ascontiguousarray(pc["gidx"]),
                "oh": np.asarray(jnp.asarray(pc["oh"], jnp.bfloat16)),
                "cmat": pc["cmat"].astype(mybir.dt.np(mybir.dt.float8e4)),
                "emb": emb_pad,
                "esh": np.ascontiguousarray(e_full[c * SLOTS : (c + 1) * SLOTS]),
                "ws": ws_bf,
                "wn": wn_bf,
                "bias": b,
            }
        )

    res = bass_utils.run_bass_kernel_spmd(
        nc, in_maps, core_ids=list(range(M)), trace=_trace
    )
    global LAST_EXEC_NS
    LAST_EXEC_NS = res.exec_time_ns
    outs = [np.asarray(r["hout"]).astype(np.float32) for r in res.results]
    big = np.concatenate(outs, axis=0)
    return big[gslot]
